# revision 11
# baseline (speedup 1.0000x reference)
"""AdaAtt attention kernel for Trainium2 (8 NeuronCores, data-parallel over batch).

v3: cuts on every engine vs v2 (255us):
  - W_ho / W_a2h stay bf16: 1-byte float weights (e3m4/e4m3) fail the 2e-2
    gate on these two (direct multiplicative path to the output), and int8
    matmul is not exposed by bass. Streams batched into 2MB transfers.
  - conv_feat packed into 28 [256, 2048] slot-row groups shared across
    batch rows (each row's 196 slots padded to 224 so every DVE segment
    write starts 32-aligned): -2.3MB DMA, -3.4us PE on the vis matmuls.
    pim holds per-group masked DoubleRow lhsT columns.
  - broadcast add (cfe + hoe[b]) moved from tensor_tensor (1x, fp8) to
    per-chunk tensor_scalar_add with an f32 [128,1] per-partition scalar:
    2x_2P DVE mode, ~25us less DVE busy.
  - softmax normalizer via DVE tensor_reduce instead of ACT accum_out
    (drops the ACTIVATION_READ_ACCUMULATOR tail, ~9us ACT).
  - biases packed into one [5, D] DMA; weight streams batched into 1MB
    transfers to cut sync-queue dispatch overhead at startup.

Self-contained: takes full inputs (as produced by the problem's setup_inputs),
shards batch across 8 cores, runs a Bass/Tile kernel via run_bass_kernel_spmd,
and returns the full [256, 2048] float32 output.
"""

from contextlib import ExitStack

import ml_dtypes
import numpy as np

import concourse.bass as bass
import concourse.mybir as mybir
import concourse.tile as tile
from concourse import bacc
from concourse.bass_utils import run_bass_kernel_spmd
from concourse.masks import make_identity

# Problem dims (hardcoded per spec)
B, A, D = 256, 196, 2048
NCORES = 8
BC = B // NCORES          # 32 batch rows per core
P = 128
KC = D // P               # 16 feature chunks
NP = KC // 2              # 8 DoubleRow chunk pairs
NS = D // 512             # 4 psum n-slices
A1 = A - P                # 68 rows in second conv chunk
AP2 = 224                 # per-row padded slot count (32-aligned segments)
G = BC * AP2 // 256       # 28 dense vis slot-row groups of 256

XS = 16.0                 # fp8 scale for inputs/activations
WS = 64.0                 # fp8-e4m3 scale for DR weights

F32 = mybir.dt.float32
BF16 = mybir.dt.bfloat16
FP8 = mybir.dt.float8e4
AFT = mybir.ActivationFunctionType
ALU = mybir.AluOpType
AXL = mybir.AxisListType
DR = mybir.MatmulPerfMode.DoubleRow

BIAS_ROW = {"bho": 0, "bhoe": 1, "bfr": 2, "bfre": 3, "ba2h": 4}

_CACHE = {}


def _vis_segments(b):
    """Static layout of batch row b's 196 exp weights inside the dense pim.

    Returns a list of (col, p0, src_col, src_row, length): copy
    tp[src_row : src_row+length, src_col] -> pim[p0 : p0+length, col].
    Global slot-row r = b*AP2 + i (i < 196) lives at group g = r//256,
    k = (r//128)%2, partition r%128; pim column for (g, k, b) is
    (2*g + k)*BC + b. src_col 0 holds erow[0:128].T, src_col 2 holds
    erow[128:196].T. AP2 % 32 == 0 keeps every start 32-aligned.
    """
    allow = {0: 128, 32: 32, 64: 64, 96: 32}  # legal SBUF AP partition spans
    r0, r1 = b * AP2, b * AP2 + A
    cuts = {r0, r1, r0 + P}
    cuts.update(r for r in range((r0 // P) * P, r1 + P, P) if r0 < r < r1)
    cs = sorted(c for c in cuts if r0 <= c <= r1)
    segs = []
    for s, e in zip(cs[:-1], cs[1:]):
        while s < e:
            g, k, p0 = s // 256, (s // P) % 2, s % P
            i0 = s - r0
            src_col, src_row = (0, i0) if i0 < P else (2, i0 - P)
            ln = min(e - s, allow[p0], allow[src_row])
            segs.append(((2 * g + k) * BC + b, p0, src_col, src_row, ln))
            s += ln
    return segs


def _build_graph():
    nc = bacc.Bacc("TRN2")

    # ---------------- DRAM parameters ----------------
    xfr_d = nc.dram_tensor("xfr", [P, KC * BC], FP8, kind="ExternalInput")   # 16*fake_region.T
    xho_d = nc.dram_tensor("xho", [P, KC * BC], BF16, kind="ExternalInput")  # h_out.T
    cfe_d = nc.dram_tensor("cfe", [BC, P, KC * A], FP8, kind="ExternalInput")  # 16*cfe
    # conv_feat packed densely: group g holds global slot-rows [256g, 256g+256)
    # where row b*224+a (a < 196) is 16*conv_feat[b, a, :], rest zero-padded.
    cf_d = nc.dram_tensor("cf", [G, P, 2 * D], FP8, kind="ExternalInput")
    w8_d = {
        name: nc.dram_tensor(name, [P, KC * D], FP8, kind="ExternalInput")     # 64*W.T
        for name in ["wfr", "wfre", "whoe"]
    }
    we_d = {
        name: nc.dram_tensor(name, [P, KC * D], BF16, kind="ExternalInput")    # W.T
        for name in ["who", "wa2h"]
    }
    # biases pre-scaled on host to match their psum scales, packed in one DMA
    bias_d = nc.dram_tensor("bias", [1, 5 * D], BF16, kind="ExternalInput")
    # w_alpha padded to 16 cols per chunk (col 0 real, rest zero): DoubleRow
    # ldweights requires k-tile step % 16 == 0
    wal_d = nc.dram_tensor("walpha", [P, KC * 16], FP8, kind="ExternalInput")  # 64*w_alpha
    out_d = nc.dram_tensor("out", [BC, D], F32, kind="ExternalOutput")

    with ExitStack() as ctx:
        tc = ctx.enter_context(tile.TileContext(nc))

        singles = ctx.enter_context(tc.tile_pool(name="singles", bufs=1))
        wpool = ctx.enter_context(tc.tile_pool(name="wpool", bufs=4))
        bmpool = ctx.enter_context(tc.tile_pool(name="bm", bufs=2))
        cfepool = ctx.enter_context(tc.tile_pool(name="cfep", bufs=4))
        thpool = ctx.enter_context(tc.tile_pool(name="thp", bufs=3))
        habpool = ctx.enter_context(tc.tile_pool(name="habp", bufs=3))
        cfpool = ctx.enter_context(tc.tile_pool(name="cfp", bufs=4))
        misc = ctx.enter_context(tc.tile_pool(name="misc", bufs=2))
        rowpool = ctx.enter_context(tc.tile_pool(name="rows", bufs=4))

        mpsum = ctx.enter_context(tc.tile_pool(name="mpsum", bufs=4, space="PSUM"))
        tpsum = ctx.enter_context(tc.tile_pool(name="tpsum", bufs=2, space="PSUM"))
        spsum = ctx.enter_context(tc.tile_pool(name="spsum", bufs=2, space="PSUM"))

        # ---------------- constants / small inputs ----------------
        bias_sb = singles.tile([1, 5 * D], BF16, tag="bias")
        nc.sync.dma_start(bias_sb[:], bias_d[:])
        xho = singles.tile([P, KC * BC], BF16, tag="xho")
        nc.sync.dma_start(xho[:], xho_d[:])

        ones = singles.tile([1, P], BF16, tag="ones")
        nc.vector.memset(ones[:], 1.0)
        id_bf = singles.tile([BC, BC], BF16, tag="id_bf")
        make_identity(nc, id_bf[:])
        id_f32 = singles.tile([BC, BC], F32, tag="id_f32")
        make_identity(nc, id_f32[:])

        def bias_ap(name):
            r = BIAS_ROW[name]
            return bias_sb[0:1, r * D : (r + 1) * D]

        # ---------------- helpers ----------------
        def linear_stream(x_lhsT, wname, bname, act, out_bm, scale=1.0):
            """out_bm[BC, D] = act((x @ W.T + b)*scale); bf16 weights
            streamed in 4-chunk (2MB) transfers, all 4 psum n-slices
            accumulating concurrently."""
            ps = [mpsum.tile([BC, 512], F32, tag="mp", name=f"mp_{wname}{ns}") for ns in range(NS)]
            for q in range(KC // 4):
                wt = wpool.tile([P, 4 * D], BF16, tag="w", name=f"w_{wname}{q}")
                nc.sync.dma_start(wt[:], we_d[wname][:, 4 * q * D : (4 * q + 4) * D])
                for kk in range(4):
                    k = 4 * q + kk
                    for ns in range(NS):
                        nc.tensor.matmul(
                            ps[ns][:],
                            lhsT=x_lhsT[:, k * BC : (k + 1) * BC],
                            rhs=wt[:, kk * D + ns * 512 : kk * D + (ns + 1) * 512],
                            start=(k == 0),
                            stop=False,
                        )
            for ns in range(NS):
                nc.tensor.matmul(
                    ps[ns][:],
                    lhsT=ones[0:1, 0:BC],
                    rhs=bias_ap(bname)[:, ns * 512 : (ns + 1) * 512],
                    start=False,
                    stop=True,
                )
                nc.scalar.activation(out_bm[:, ns * 512 : (ns + 1) * 512], ps[ns][:], act, scale=scale)
            return out_bm

        def linear_dr(x8_lhsT, wname, bname, act, out_bm, scale):
            """out_bm[BC, D] = act((psum + b)*scale) with fp8 DoubleRow matmuls.
            x8_lhsT [P, KC*BC] fp8, weight [P, KC*D] fp8; psum = xscale*wscale*(x@W.T)."""
            ps = [mpsum.tile([BC, 512], F32, tag="mp", name=f"mp_{wname}{ns}") for ns in range(NS)]
            for q in range(NP // 2):
                wt = wpool.tile([P, 4 * D], FP8, tag="w", name=f"w_{wname}{q}")
                nc.sync.dma_start(wt[:], w8_d[wname][:, 4 * q * D : (4 * q + 4) * D])
                for pp in range(2):
                    p = 2 * q + pp
                    wv = wt[:, pp * 2 * D : (pp + 1) * 2 * D].rearrange(
                        "q (k n) -> q k n", n=D
                    )
                    xv = x8_lhsT[:, 2 * p * BC : (2 * p + 2) * BC].rearrange(
                        "q (k m) -> q k m", m=BC
                    )
                    for ns in range(NS):
                        nc.tensor.matmul(
                            ps[ns][:],
                            lhsT=xv,
                            rhs=wv[:, :, ns * 512 : (ns + 1) * 512],
                            start=(p == 0),
                            stop=False,
                            perf_mode=DR,
                        )
            for ns in range(NS):
                nc.tensor.matmul(
                    ps[ns][:],
                    lhsT=ones[0:1, 0:BC],
                    rhs=bias_ap(bname)[:, ns * 512 : (ns + 1) * 512],
                    start=False,
                    stop=True,
                )
                nc.scalar.activation(out_bm[:, ns * 512 : (ns + 1) * 512], ps[ns][:], act, scale=scale)
            return out_bm

        def to_feature_major(bm, outs, in_f32=False):
            """bm [BC, D] -> feature-major [P, KC*BC] via PE transposes.
            outs: list of (tile, mul) to produce (mul applied on DVE, dtype
            conversion via the copy)."""
            ident = id_f32 if in_f32 else id_bf
            dt = F32 if in_f32 else BF16
            for k in range(KC):
                pt = tpsum.tile([P, BC], dt, tag="tps", name=f"pt_{outs[0][0].name}{k}")
                nc.tensor.transpose(pt[:], bm[:, k * P : (k + 1) * P], ident[:])
                for t, mul in outs:
                    if mul == 1.0:
                        nc.vector.tensor_copy(t[:, k * BC : (k + 1) * BC], pt[:])
                    else:
                        nc.vector.tensor_scalar_mul(
                            t[:, k * BC : (k + 1) * BC], pt[:], mul
                        )

        # ---------------- phase 1: front linears ----------------
        # ho-chain first: hoeT/hoe16T are the only inputs the fused attention
        # loop needs.
        hol_bm = bmpool.tile([BC, D], BF16, tag="bm", name="hol_bm")
        linear_stream(xho, "who", "bho", AFT.Tanh, hol_bm)
        holT = singles.tile([P, KC * BC], BF16, tag="holT")
        hol8T = singles.tile([P, KC * BC], FP8, tag="hol8T")   # 16*hol
        to_feature_major(hol_bm, [(holT, 1.0), (hol8T, XS)])

        hoe_bm = bmpool.tile([BC, D], BF16, tag="bm", name="hoe_bm")
        linear_dr(hol8T, "whoe", "bhoe", AFT.Copy, hoe_bm, scale=1.0 / (XS * WS))
        hoeT = singles.tile([P, KC * BC], BF16, tag="hoeT")
        hoe16T = singles.tile([P, KC * BC], F32, tag="hoe16T")       # 16*hoe
        to_feature_major(hoe_bm, [(hoeT, 1.0), (hoe16T, XS)])

        xfr = singles.tile([P, KC * BC], FP8, tag="xfr")
        nc.sync.dma_start(xfr[:], xfr_d[:])
        wal = singles.tile([P, KC * 16], FP8, tag="wal")
        nc.sync.dma_start(wal[:], wal_d[:])

        frT8 = singles.tile([P, KC * BC], FP8, tag="frT8")           # 16*fr
        freT = singles.tile([P, KC * BC], BF16, tag="freT")
        e0all = singles.tile([1, BC], F32, tag="e0all")

        def fr_chain():
            """fr/fre linears + slot-0 score; emitted mid-loop so their weight
            streams overlap the attention loop instead of delaying it."""
            fr_bm = bmpool.tile([BC, D], BF16, tag="bm", name="fr_bm")   # 16*fr
            linear_dr(xfr, "wfr", "bfr", AFT.Relu, fr_bm, scale=1.0 / WS)
            to_feature_major(fr_bm, [(frT8, 1.0)])

            fre_bm = bmpool.tile([BC, D], BF16, tag="bm", name="fre_bm")
            linear_dr(frT8, "wfre", "bfre", AFT.Copy, fre_bm, scale=1.0 / (XS * WS))
            to_feature_major(fre_bm, [(freT, 1.0)])

            # slot-0 scores for all b: w_alpha . tanh(fre + hoe)
            ha0 = misc.tile([P, KC * BC], BF16, tag="ha0")
            nc.vector.tensor_tensor(ha0[:], freT[:], hoeT[:], op=ALU.add)
            ta0 = misc.tile([P, KC * BC], FP8, tag="ta0")
            nc.scalar.activation(ta0[:], ha0[:], AFT.Tanh)
            s0ps = spsum.tile([1, A], F32, tag="sps", name="s0ps")
            for c in range(KC):
                nc.tensor.matmul(
                    s0ps[0:1, 0:BC],
                    lhsT=wal[:, 16 * c : 16 * c + 1],
                    rhs=ta0[:, c * BC : (c + 1) * BC],
                    start=(c == 0),
                    stop=(c == KC - 1),
                )
            # slot-0 exp weights (scores bounded, no max subtraction needed)
            nc.scalar.activation(e0all[:], s0ps[0:1, 0:BC], AFT.Exp, scale=1.0 / WS)

        # ---------------- phase 2 (fused): scores -> row softmax -> vis ----------------
        # pim holds, per dense slot-row group g, a [128, 2, 32] fp8 block whose
        # column b (in the k-tile covering b's slot rows) is 16*exp(score);
        # accumulating all 25 groups into shared [32, 512] psum tiles via
        # DoubleRow matmuls yields 256*unnormalized vis for every batch row.
        pim = singles.tile([P, G * 2 * BC], FP8, tag="pim")
        nc.vector.memset(pim[:], 0.0)
        Zrow = singles.tile([1, BC], F32, tag="Zrow")

        # vp allocated lazily at the first emit_vis (b==2) so fr_chain's
        # psum tiles (emitted at b==1) don't collide with it in the pool
        vp = []
        cf_tiles = {}
        next_dma_g = 0
        emitted_g = 0

        def emit_vis(g):
            if not vp:
                vp.extend(
                    mpsum.tile([BC, 512], F32, tag="mp", name=f"vp{ns}")
                    for ns in range(NS)
                )
            c01 = cf_tiles.pop(g)
            cv = c01[:].rearrange("q (k n) -> q k n", n=D)
            pv = pim[:, g * 2 * BC : (g + 1) * 2 * BC].rearrange(
                "q (k m) -> q k m", m=BC
            )
            for ns in range(NS):
                nc.tensor.matmul(
                    vp[ns][:],
                    lhsT=pv,
                    rhs=cv[:, :, ns * 512 : (ns + 1) * 512],
                    start=(g == 0),
                    stop=(g == G - 1),
                    perf_mode=DR,
                )

        # group g's pim columns are complete once batch row hi_b(g) has
        # written its exp weights
        hi_b = [min(BC - 1, (256 * (g + 1) - 1) // AP2) for g in range(G)]
        NH = 2
        HK = KC // NH
        wa2h_pre = []
        for b in range(BC):
            cfeb = cfepool.tile([P, KC * A], FP8, tag="cfeb")
            nc.sync.dma_start(cfeb[:], cfe_d[b])
            # prefetch dense conv_feat groups (pool-gated, 4 in flight)
            while next_dma_g < G and next_dma_g < emitted_g + 4:
                t = cfpool.tile([P, 2 * D], FP8, tag="c01", name=f"cf_g{next_dma_g}")
                nc.sync.dma_start(t[:], cf_d[next_dma_g])
                cf_tiles[next_dma_g] = t
                next_dma_g += 1

            # th = tanh(cfe + hoe[b]): per-chunk tensor_scalar_add (2x_2P DVE
            # mode, f32 per-partition scalar), tanh on ACT in two halves
            hab = habpool.tile([P, KC * A], FP8, tag="hab")
            th = thpool.tile([P, KC * A], FP8, tag="th")
            for h in range(NH):
                for cc in range(HK):
                    c = h * HK + cc
                    nc.vector.tensor_scalar_add(
                        hab[:, c * A : (c + 1) * A],
                        cfeb[:, c * A : (c + 1) * A],
                        hoe16T[:, c * BC + b : c * BC + b + 1],
                    )
                sl = slice(h * HK * A, (h + 1) * HK * A)
                nc.scalar.activation(th[:, sl], hab[:, sl], AFT.Tanh, scale=1.0 / XS)
            sps = spsum.tile([16, A], F32, tag="sps")
            for c in range(NP):
                wv = wal[:, 32 * c : 32 * (c + 1)].rearrange("q (k o) -> q k o", o=16)
                tv = th[:, 2 * c * A : (2 * c + 2) * A].rearrange(
                    "q (k a) -> q k a", a=A
                )
                nc.tensor.matmul(
                    sps[:],
                    lhsT=wv,
                    rhs=tv,
                    start=(c == 0),
                    stop=(c == NP - 1),
                    perf_mode=DR,
                )
            # emit vis matmuls for groups completed by b-1 (one-iteration
            # pipeline delay keeps the PE fed while b's softmax finishes);
            # first emit at b==2 so fr_chain's psum frees before vp allocates
            if b >= 2:
                while emitted_g < G and hi_b[emitted_g] <= b - 1:
                    emit_vis(emitted_g)
                    emitted_g += 1
            # unnormalized exp weights straight from the score psum (scores
            # bounded, exp safe); Z row-sum on DVE
            erow = rowpool.tile([1, A], BF16, tag="erow")
            nc.scalar.activation(erow[:], sps[0:1, :], AFT.Exp, scale=1.0 / WS)
            nc.vector.tensor_reduce(
                Zrow[0:1, b : b + 1], erow[:], axis=AXL.X, op=ALU.add
            )
            # transpose exp weights and scatter into the dense pim columns (x16)
            tp = tpsum.tile([P, 4], BF16, tag="tps", name=f"tp{b}")
            nc.tensor.transpose(tp[:, 0:1], erow[0:1, 0:P], id_bf[0:1, 0:1])
            nc.tensor.transpose(tp[0:A1, 2:3], erow[0:1, P:A], id_bf[0:1, 0:1])
            for col, p0, sc, sr, ln in _vis_segments(b):
                nc.vector.tensor_scalar_mul(
                    pim[p0 : p0 + ln, col : col + 1], tp[sr : sr + ln, sc : sc + 1], XS
                )
            if b == 1:
                fr_chain()
            # prefetch ALL final-layer weight quads late in the loop
            if BC - 10 <= b < BC - 10 + KC // 4:
                q = b - (BC - 10)
                wt = wpool.tile([P, 4 * D], BF16, tag="w", name=f"w_a2h{q}")
                nc.sync.dma_start(wt[:], we_d["wa2h"][:, 4 * q * D : (4 * q + 4) * D])
                wa2h_pre.append(wt)
        while emitted_g < G:
            emit_vis(emitted_g)
            emitted_g += 1

        # ---------------- phase 3: normalize, atten_out, final linear ----------------
        # total Z = conv-slot sum + slot-0 exp; 1/(256 Z) as a column for vis rows
        nc.vector.tensor_tensor(Zrow[:], Zrow[:], e0all[:], op=ALU.add)
        zt = tpsum.tile([P, 2], F32, tag="tps", name="zt")
        nc.tensor.transpose(zt[0:BC, 0:1], Zrow[:], id_f32[0:1, 0:1])
        zcol = singles.tile([BC, 1], F32, tag="zcol")
        nc.vector.tensor_scalar_mul(zcol[:], zt[0:BC, 0:1], XS * XS)
        rinv = singles.tile([BC, 1], F32, tag="rinv")
        nc.vector.reciprocal(rinv[:], zcol[:])           # 1/(256 Z)
        vis_bm = singles.tile([BC, D], F32, tag="vis_bm")
        for ns in range(NS):
            nc.vector.tensor_scalar_mul(
                vis_bm[:, ns * 512 : (ns + 1) * 512], vp[ns][:], rinv[:]
            )
        # pi0/16 = e0/(16 Z) broadcast to [128, BC] (the 1/16 cancels frT8's x16)
        zs = singles.tile([1, BC], F32, tag="zs")
        nc.vector.tensor_scalar_mul(zs[:], Zrow[:], XS)
        zrinv = singles.tile([1, BC], F32, tag="zrinv")
        nc.vector.reciprocal(zrinv[:], zs[:])
        e0z = singles.tile([1, BC], BF16, tag="e0z")
        nc.vector.tensor_tensor(e0z[:], e0all[:], zrinv[:], op=ALU.mult)
        pb = tpsum.tile([P, BC], F32, tag="tps", name="pb")
        nc.tensor.matmul(pb[:], lhsT=ones[0:1, 0:P], rhs=e0z[:], start=True, stop=True)
        pi0b = singles.tile([P, BC], F32, tag="pi0b")
        nc.vector.tensor_copy(pi0b[:], pb[:])

        # atten_out.T = vis.T + hol.T + (pi0/16)*(16 fr.T)  (feature-major, bf16)
        attT = singles.tile([P, KC * BC], BF16, tag="attT")
        for k in range(KC):
            vt = tpsum.tile([P, BC], F32, tag="tps", name=f"vt{k}")
            nc.tensor.transpose(vt[:], vis_bm[:, k * P : (k + 1) * P], id_f32[:])
            t1 = misc.tile([P, BC], F32, tag="t1")
            nc.vector.tensor_tensor(t1[:], vt[:], holT[:, k * BC : (k + 1) * BC], op=ALU.add)
            t2 = misc.tile([P, BC], F32, tag="t2")
            nc.vector.tensor_tensor(t2[:], pi0b[:], frT8[:, k * BC : (k + 1) * BC], op=ALU.mult)
            nc.vector.tensor_tensor(
                attT[:, k * BC : (k + 1) * BC], t1[:], t2[:], op=ALU.add
            )

        # final linear: out = tanh(atten @ W_a2h.T + b), e3m4 weights
        ps = [mpsum.tile([BC, 512], F32, tag="mp", name=f"fps{ns}") for ns in range(NS)]
        for q in range(KC // 4):
            wt = wa2h_pre[q]
            for kk in range(4):
                k = 4 * q + kk
                for ns in range(NS):
                    nc.tensor.matmul(
                        ps[ns][:],
                        lhsT=attT[:, k * BC : (k + 1) * BC],
                        rhs=wt[:, kk * D + ns * 512 : kk * D + (ns + 1) * 512],
                        start=(k == 0),
                        stop=False,
                    )
        outsb = singles.tile([BC, D], F32, tag="outsb")
        for ns in range(NS):
            nc.tensor.matmul(
                ps[ns][:],
                lhsT=ones[0:1, 0:BC],
                rhs=bias_ap("ba2h")[:, ns * 512 : (ns + 1) * 512],
                start=False,
                stop=True,
            )
            nc.scalar.activation(outsb[:, ns * 512 : (ns + 1) * 512], ps[ns][:], AFT.Tanh)
        nc.sync.dma_start(out_d[:], outsb[:])

    nc.compile()
    return nc


def _bf16(x):
    return np.ascontiguousarray(np.asarray(x, dtype=np.float32).astype(ml_dtypes.bfloat16))


def _fp8(x):
    return np.ascontiguousarray(np.asarray(x, dtype=np.float32).astype(ml_dtypes.float8_e4m3))


def _chunked_wT(W):
    # W [D, D] (out_features, in_features) -> W.T chunk layout [128, KC*D] f32
    Wt = np.asarray(W, dtype=np.float32).T  # [k, n]
    return Wt.reshape(KC, P, D).transpose(1, 0, 2).reshape(P, KC * D)


def _prep_xT(x, scale):
    # x [BC, D] -> X.T chunk layout [128, KC*BC] f32
    t = np.asarray(x, dtype=np.float32).T * scale  # [D, BC]
    return t.reshape(KC, P, BC).transpose(1, 0, 2).reshape(P, KC * BC)


def _prep_cfe(e):
    # e [BC, A, D] -> [BC, 128, KC*A] with chunk c at free offset c*A, fp8 (x16)
    t = np.asarray(e, dtype=np.float32).transpose(0, 2, 1) * XS  # [BC, D, A]
    r = t.reshape(BC, KC, P, A).transpose(0, 2, 1, 3).reshape(BC, P, KC * A)
    return _fp8(r)


def _prep_cf(cf):
    # cf [BC, A, D] -> [G, 128, 2*D] fp8 (x16): dense slot-row groups, group g
    # row p, half k holds flat row 256g+128k+p where flat row b*AP2+a is
    # cf[b, a] for a < 196, zero in the 28-row per-b pad
    t = np.zeros((BC, AP2, D), np.float32)
    t[:, :A] = np.asarray(cf, dtype=np.float32) * XS
    t = t.reshape(G, 2, P, D).transpose(0, 2, 1, 3).reshape(G, P, 2 * D)
    return _fp8(t)


def _prep_walpha(w):
    # [D] -> [128, KC*16] fp8 (x64): chunk k lives in col 16k, rest zero
    wc = np.asarray(w, dtype=np.float32).reshape(KC, P).T * WS  # [128, KC]
    out = np.zeros((P, KC * 16), dtype=np.float32)
    out[:, ::16] = wc
    return _fp8(out)


def _make_in_maps(inputs):
    h_out = np.asarray(inputs["h_out"], dtype=np.float32)
    fake_region = np.asarray(inputs["fake_region"], dtype=np.float32)
    conv_feat = np.asarray(inputs["conv_feat"], dtype=np.float32)
    conv_feat_embed = np.asarray(inputs["conv_feat_embed"], dtype=np.float32)

    bias_pack = np.zeros((5, D), dtype=np.float32)
    # packed along the free dim: row r lives at [0, r*D : (r+1)*D]
    bias_pack[BIAS_ROW["bho"]] = np.asarray(inputs["b_ho"], np.float32)
    bias_pack[BIAS_ROW["bhoe"]] = np.asarray(inputs["b_hoe"], np.float32) * (XS * WS)
    bias_pack[BIAS_ROW["bfr"]] = np.asarray(inputs["b_fr"], np.float32) * (XS * WS)
    bias_pack[BIAS_ROW["bfre"]] = np.asarray(inputs["b_fre"], np.float32) * (XS * WS)
    bias_pack[BIAS_ROW["ba2h"]] = np.asarray(inputs["b_a2h"], np.float32)

    shared = {
        "wfr": _fp8(_chunked_wT(inputs["W_fr"]) * WS),
        "wfre": _fp8(_chunked_wT(inputs["W_fre"]) * WS),
        "whoe": _fp8(_chunked_wT(inputs["W_hoe"]) * WS),
        "who": _bf16(_chunked_wT(inputs["W_ho"])),
        "wa2h": _bf16(_chunked_wT(inputs["W_a2h"])),
        "bias": _bf16(bias_pack.reshape(1, 5 * D)),
        "walpha": _prep_walpha(inputs["w_alpha"]),
    }
    in_maps = []
    for c in range(NCORES):
        sl = slice(c * BC, (c + 1) * BC)
        in_maps.append(
            dict(
                shared,
                xfr=_fp8(_prep_xT(fake_region[sl], XS)),
                xho=_bf16(_prep_xT(h_out[sl], 1.0)),
                cfe=_prep_cfe(conv_feat_embed[sl]),
                cf=_prep_cf(conv_feat[sl]),
            )
        )
    return in_maps


def _run(inputs, trace=False):
    if "nc" not in _CACHE:
        _CACHE["nc"] = _build_graph()
    nc = _CACHE["nc"]
    in_maps = _make_in_maps(inputs)
    res = run_bass_kernel_spmd(nc, in_maps, core_ids=list(range(NCORES)), trace=trace)
    out = np.concatenate([r["out"] for r in res.results], axis=0)
    return out.astype(np.float32), res


def kernel(**inputs):
    out, _ = _run(inputs, trace=False)
    return out
